# revision 83
# baseline (speedup 1.0000x reference)
"""Trainium2 Bass kernel for a decoder block (LN -> MHA -> LN -> FFN).

Sharding: heads across the 8 cores for attention (2 heads/core), tokens
across cores for dense/LN2/FFN (512 tokens/core), connected by an
AllToAll of the softmax-normalized ctx in bf16 — split into one
collective per batch so the first overlaps batch-1 attention.

All matmuls run in bf16 with fp32 PSUM accumulation; layernorm stats,
softmax input, and residuals stay fp32.  gamma1/beta1 are folded into
the QKV weights/biases on the host, gamma2/beta2 into fc_w and a fc
output bias; the true (gamma,beta)-applied xn / hn needed for the
residual connections are computed on-chip for the core's own token
shard only.

Engine budget notes:
 - x ships as bf16 (halves DMA, enables the 4x DVE tensor_scalar mode
   for the LN normalize); weight tensors are host-relayout'd so every
   DMA row is >= 512B (avoids the small-descriptor 2x penalty).
 - rstd = exp(-0.5*ln(c*var)): keeps every phase-1 activation in the
   natural_log_exp_and_others table set (one pre-placed load, no
   mid-kernel ACT table switches; Gelu loads once in phase 2).
 - causal mask lands in PSUM via an identity-matmul prefill of the
   shared triangle block; the diagonal scores matmul accumulates onto
   it (no element-wise mask adds).
 - softmax exp runs once per k-tile over both heads' scores; the
   next macro's LN-finish + QKV are emitted between scores and ctx so
   the PE stays busy while ACT drains the exp backlog.
 - softmax denominator: reciprocal row is broadcast to rows 64:128 of
   the ctx PSUM bank by a ones-matmul, staged through SBUF for the
   normalize multiply (HW allows only one PSUM operand per DVE op).
 - the batch-0 A2A is issued mid-phase-1; the batch-1 A2A is bridged
   by dense/LN2/fc work on the batch-0 token half (fc weights for the
   first NRES rows stay resident so no DMA sits on that path).
 - proj weights stream in 8 k-tile chunks through a 6-slot ring,
   interleaved with the fc weight stream; the proj j-order is rotated
   per token tile so the ring reuse lands just in time.
"""

from contextlib import ExitStack

import numpy as np
import ml_dtypes

B, S, D = 2, 2048, 1024
H, DEP = 16, 64
NT = B * S            # 4096 flattened tokens
NCORES = 8
HPC = H // NCORES     # 2 heads per core
TPC = NT // NCORES    # 512 tokens per core
QM = 512              # q-macro / token-macro size
KT = 128              # k-tile size
NEG = -1.0e9

_cache = {}


def _build_program():
    import concourse.bacc as bacc
    import concourse.tile as tile
    import concourse.mybir as mybir
    from concourse.masks import make_identity
    from concourse.hw_specs import get_activation_tables

    dt = mybir.dt
    AF = mybir.ActivationFunctionType
    OP = mybir.AluOpType

    nc = bacc.Bacc("TRN2", target_bir_lowering=False, debug=False,
                   num_devices=NCORES)

    def din(name, shape, dtype=dt.float32):
        return nc.dram_tensor(name, shape, dtype, kind="ExternalInput").ap()

    x_full = din("x_full", [NT, D], dt.bfloat16)
    x_shard = din("x_shard", [TPC, D], dt.bfloat16)
    wqt = din("wqt", [128, 8, 128], dt.bfloat16)
    wkt = din("wkt", [128, 8, 128], dt.bfloat16)
    wvt = din("wvt", [128, 8, 128], dt.bfloat16)
    qb_i = din("qb", [128, 1])
    kb_i = din("kb", [128, 1])
    vb_i = din("vb", [128, 1])
    g1b_i = din("g1b", [128, D])
    b1b_i = din("b1b", [128, D])
    g2b_i = din("g2b", [128, D])
    b2b_i = din("b2b", [128, D])
    dense_wt = din("dense_wt", [D, D], dt.bfloat16)
    fc_wt = din("fc_wt", [32, 128, 8, 128], dt.bfloat16)
    fcb_i = din("fcb", [128, 32])
    proj_wt = din("proj_wt", [4 * D, D], dt.bfloat16)
    mask_i = din("mask_tri", [128, 128], dt.bfloat16)
    out_sh = nc.dram_tensor("out_shard", [TPC, D], dt.float32,
                            kind="ExternalOutput").ap()

    VAR_SCALE = float(D) / float(D - 1)   # ddof=1 correction

    set_names = [t[0] for t in get_activation_tables(nc.m.arch).items()]
    NLE_SET = set_names.index("natural_log_exp_and_others")

    with tile.TileContext(nc) as tc:
        with ExitStack() as es0:
            consts = es0.enter_context(tc.tile_pool(name="consts", bufs=1))
            dram = es0.enter_context(
                tc.tile_pool(name="dram", bufs=1, space="DRAM"))
            # pre-place the ln+exp table so phase 1 never switches sets
            nc.scalar.add_instruction(mybir.InstLoadActFuncSet(
                name=nc.get_next_instruction_name(),
                act_func_set_id=NLE_SET, ins=[], outs=[]))
            ident_bf = consts.tile([128, 128], dt.bfloat16)
            make_identity(nc, ident_bf)
            ones_bf = consts.tile([1, 64], dt.bfloat16)
            nc.vector.memset(ones_bf, 1.0)
            ident_f32 = consts.tile([128, 128], dt.float32)
            make_identity(nc, ident_f32)
            qb = consts.tile([128, 1], dt.float32)
            kb = consts.tile([128, 1], dt.float32)
            vb = consts.tile([128, 1], dt.float32)
            mask_tri = consts.tile([128, 128], dt.bfloat16)

            xnsh_pool = es0.enter_context(tc.tile_pool(name="xnsh", bufs=1))
            xn_sh = xnsh_pool.tile([128, 4, D], dt.float32)
            p2pre = es0.enter_context(tc.tile_pool(name="p2pre", bufs=1))
            ctxT = p2pre.tile([128, 8, TPC], dt.bfloat16)
            dense_sb0 = p2pre.tile([128, 8, QM], dt.bfloat16)

            # batch-mixed shards: core c owns 256 tokens of each batch
            # (batch b, macro c//2, half c%2), so both A2As carry only
            # real data and land in disjoint token ranges of ctxT.
            a2a_in = [dram.tile([NCORES, HPC, DEP, QM // 2], dt.bfloat16,
                                name=f"a2a_in{bb}") for bb in range(2)]
            a2a_out = [dram.tile([NCORES, HPC, DEP, QM // 2], dt.bfloat16,
                                 name=f"a2a_out{bb}") for bb in range(2)]
            resh = lambda t: t.rearrange("c h d q -> (c h d) q").rearrange(
                "(k p) q -> p k q", p=128)

            # ------- LN1 + QKV interleaved, then attention per batch -------
            with ExitStack() as es1:
                P = lambda *a, **k: es1.enter_context(tc.tile_pool(*a, **k))
                xt_pool = P(name="xt", bufs=2)
                st_pool = P(name="stats", bufs=3)
                xnT_pool = P(name="xnT", bufs=1)
                qkT_pool = P(name="qkT", bufs=1)
                v_pool = P(name="vtok", bufs=1)
                wq_pool = P(name="wq", bufs=1)
                ps_tr = P(name="ps_tr", bufs=1, space="PSUM")
                ps_sc = P(name="ps_sc", bufs=2, space="PSUM")
                ps_qk = P(name="ps_qk", bufs=1, space="PSUM")
                pr_pool = P(name="probs", bufs=16)
                psctx = P(name="psctx", bufs=2, space="PSUM")
                rb_pool = P(name="rbp", bufs=2)

                xn_T = xnT_pool.tile([128, 8, NT], dt.bfloat16)
                q_T = qkT_pool.tile([128, NT], dt.bfloat16)
                k_T = qkT_pool.tile([128, NT], dt.bfloat16)
                vtok = v_pool.tile([128, 32, 130], dt.bfloat16)
                nc.vector.memset(vtok[:, :, 64:65], 1.0)
                nc.vector.memset(vtok[:, :, 129:130], 1.0)

                ps_trT = ps_tr.tile([128, 8, 128], dt.bfloat16)
                wq_sb = wq_pool.tile([128, 8, 128], dt.bfloat16)
                wk_sb = wq_pool.tile([128, 8, 128], dt.bfloat16)
                wv_sb = wq_pool.tile([128, 8, 128], dt.bfloat16)
                g1b = wq_pool.tile([128, D], dt.float32)
                b1b = wq_pool.tile([128, D], dt.float32)

                def ln_stats(src, base_row):
                    """Stats for 4 consecutive 128-row tiles; per-tile rstd
                    via exp(-0.5*ln(c*var)) (stays in the ln/exp table set).
                    Returns list of (x_t, mean_ap, rstd_ap)."""
                    mv4 = st_pool.tile([128, 4, 2], dt.float32, tag="mv4")
                    rstd4 = st_pool.tile([128, 4], dt.float32, tag="rstd4")
                    xts = []
                    for i in range(4):
                        x_t = xt_pool.tile([128, D], dt.bfloat16, tag="xt",
                                           bufs=5)
                        r0 = base_row + 128 * i
                        nc.sync.dma_start(out=x_t, in_=src[r0:r0 + 128, :])
                        stats = st_pool.tile([128, 2, 6], dt.float32,
                                             tag="bnst")
                        nc.vector.bn_stats(out=stats[:, 0, :],
                                           in_=x_t[:, 0:512])
                        nc.vector.bn_stats(out=stats[:, 1, :],
                                           in_=x_t[:, 512:1024])
                        nc.vector.bn_aggr(out=mv4[:, i, :], in_=stats)
                        lnv = st_pool.tile([128, 1], dt.float32, tag="lnv")
                        nc.scalar.activation(out=lnv, in_=mv4[:, i, 1:2],
                                             func=AF.Ln, scale=VAR_SCALE)
                        nc.scalar.activation(out=rstd4[:, i:i + 1], in_=lnv,
                                             func=AF.Exp, scale=-0.5)
                        xts.append(x_t)
                    return [(xts[i], mv4[:, i, 0:1], rstd4[:, i:i + 1])
                            for i in range(4)]

                def ln_finish(m, stats, on_act=False):
                    """normalize + transpose + copy into xn_T for macro m.
                    Copies go to ACT in windows where DVE is the critical
                    engine (small attention macros), else DVE."""
                    for i, (x_t, mean, rstd) in enumerate(stats):
                        t = 4 * m + i
                        xnb = xt_pool.tile([128, D], dt.bfloat16, tag="xnb",
                                           bufs=2)
                        nc.vector.tensor_scalar(out=xnb, in0=x_t, scalar1=mean,
                                                scalar2=rstd, op0=OP.subtract,
                                                op1=OP.mult)
                        for half in range(2):
                            ps = ps_trT[:, 4 * half:4 * half + 4, :]
                            for s2 in range(4):
                                kc = 4 * half + s2
                                nc.tensor.transpose(
                                    ps[:, s2, :],
                                    xnb[:, 128 * kc:128 * (kc + 1)], ident_bf)
                            dst = xn_T[:, 4 * half:4 * half + 4,
                                       128 * t:128 * (t + 1)]
                            if on_act:
                                nc.scalar.copy(out=dst, in_=ps)
                            else:
                                nc.vector.tensor_copy(out=dst, in_=ps)

                def qkv_macro(m):
                    # bias adds go to DVE when this emission runs inside a
                    # long-exp window (ACT queue is the pacer there)
                    dve_adds = m in (7,)
                    tok = slice(QM * m, QM * (m + 1))
                    for w_sb, bias, dst in ((wq_sb, qb, q_T), (wk_sb, kb, k_T)):
                        ps = ps_qk.tile([128, QM], dt.float32, tag="qk")
                        for kc in range(8):
                            nc.tensor.matmul(ps, w_sb[:, kc, :],
                                             xn_T[:, kc, tok],
                                             start=(kc == 0), stop=(kc == 7))
                        if dve_adds:
                            nc.vector.tensor_scalar_add(dst[:, tok], ps, bias)
                        else:
                            nc.scalar.add(out=dst[:, tok], in_=ps, add=bias)
                    ps = ps_qk.tile([128, QM], dt.float32, tag="qk")
                    for kc in range(8):
                        nc.tensor.matmul(ps, wv_sb[:, kc, :], xn_T[:, kc, tok],
                                         start=(kc == 0), stop=(kc == 7))
                    vst = rb_pool.tile([128, QM], dt.bfloat16, tag="vst",
                                       bufs=2)
                    if dve_adds:
                        nc.vector.tensor_scalar_add(vst, ps, vb)
                    else:
                        nc.scalar.add(out=vst, in_=ps, add=vb)
                    for half in range(2):
                        pt = ps_trT[:, 2 * half:2 * half + 2, :]
                        for s2 in range(2):
                            s = 2 * half + s2
                            nc.tensor.transpose(
                                pt[:, s2, :], vst[:, 128 * s:128 * (s + 1)],
                                ident_bf)
                        for s2 in range(2):
                            kt_idx = 4 * m + 2 * half + s2
                            dst = vtok[:, kt_idx, 0:130].rearrange(
                                "p (two h) -> p two h", two=2)[:, :, 0:64]
                            src = pt[:, s2, :].rearrange(
                                "p (two h) -> p two h", two=2)
                            nc.vector.tensor_copy(out=dst, in_=src)

                def attention_macro(b, mm, mid_cb=None):
                    q0 = 2048 * b + QM * mm
                    nkt = 4 * mm + 4
                    probs = []
                    for j in range(nkt):
                        rel = j - 4 * mm
                        lo = 128 * rel if rel > 0 else 0
                        ks = slice(2048 * b + KT * j,
                                   2048 * b + KT * (j + 1))
                        ps = ps_sc.tile([128, 2, QM], dt.float32, tag="sc")
                        pb = pr_pool.tile([128, 2, QM], dt.bfloat16, tag="pr")
                        for h in range(2):
                            hp = slice(64 * h, 64 * (h + 1))
                            if rel >= 0:
                                hi = lo + 128
                                nc.tensor.matmul(
                                    ps[:, h, lo:hi], ident_bf, mask_tri,
                                    start=True, stop=False)
                                nc.tensor.matmul(
                                    ps[:, h, lo:hi], k_T[hp, ks],
                                    q_T[hp, q0 + lo:q0 + hi],
                                    start=False, stop=True)
                                if hi < QM:
                                    nc.tensor.matmul(
                                        ps[:, h, hi:QM], k_T[hp, ks],
                                        q_T[hp, q0 + hi:q0 + QM],
                                        start=True, stop=True)
                            else:
                                nc.tensor.matmul(
                                    ps[:, h, :], k_T[hp, ks],
                                    q_T[hp, q0:q0 + QM],
                                    start=True, stop=True)
                        nc.scalar.activation(out=pb[:, :, lo:QM],
                                             in_=ps[:, :, lo:QM],
                                             func=AF.Exp, scale=0.125)
                        probs.append((pb, lo))
                    if mid_cb is not None:
                        # emitted between scores/exp and ctx: the next
                        # macro's LN work lands on DVE/PE while ACT chews
                        # through this macro's exps
                        mid_cb()
                    pcs = []
                    for h in range(2):
                        pc = psctx.tile([128, QM], dt.float32, tag="ctx")
                        for j in range(nkt):
                            pb, lo = probs[j]
                            nc.tensor.matmul(
                                pc[0:65, lo:QM],
                                vtok[:, 16 * b + j, 65 * h:65 * (h + 1)],
                                pb[:, h, lo:QM],
                                start=(j == 0), stop=(j == nkt - 1))
                        pcs.append(pc)

                    def fin_bc(h):
                        # denominator: reciprocal row broadcast to rows
                        # 64:128 of the ctx PSUM bank via a ones-matmul,
                        # staged through SBUF (HW allows only one PSUM
                        # operand on the DVE multiply)
                        pc = pcs[h]
                        r32 = rb_pool.tile([1, QM], dt.float32, tag="r32")
                        nc.vector.reciprocal(out=r32, in_=pc[64:65, :])
                        rbf = rb_pool.tile([1, QM], dt.bfloat16, tag="rbf")
                        nc.vector.tensor_copy(out=rbf, in_=r32)
                        nc.tensor.matmul(pc[64:128, :], ones_bf, rbf,
                                         start=True, stop=True)
                        rb = rb_pool.tile([64, QM], dt.bfloat16, tag="rb",
                                          name=f"rb{h}")
                        nc.scalar.copy(out=rb, in_=pc[64:128, :])
                        return rb

                    def fin_csb(h, rb):
                        csb = rb_pool.tile([64, QM], dt.bfloat16, tag="csb")
                        nc.vector.tensor_tensor(out=csb, in0=pcs[h][0:64, :],
                                                in1=rb, op=OP.mult)
                        for hf in range(2):
                            nc.sync.dma_start(
                                out=a2a_in[b][2 * mm + hf, h],
                                in_=csb[:, 256 * hf:256 * (hf + 1)])

                    rb0 = fin_bc(0)
                    rb1 = fin_bc(1)
                    fin_csb(0, rb0)
                    fin_csb(1, rb1)

                # first macro's x tiles go down the DMA queue first, then
                # the weights/consts needed a few microseconds later
                stats_cur = ln_stats(x_full, 0)
                nc.sync.dma_start(out=wq_sb, in_=wqt)
                nc.sync.dma_start(out=qb, in_=qb_i)
                nc.sync.dma_start(out=wk_sb, in_=wkt)
                nc.sync.dma_start(out=kb, in_=kb_i)
                nc.sync.dma_start(out=wv_sb, in_=wvt)
                nc.sync.dma_start(out=vb, in_=vb_i)
                nc.sync.dma_start(out=mask_tri, in_=mask_i)
                ln_finish(0, stats_cur, on_act=True)

                def shard_finish(stats):
                    for i, (x_t, mean, rstd) in enumerate(stats):
                        xr = xt_pool.tile([128, D], dt.float32, tag="xr",
                                          bufs=2)
                        nc.vector.tensor_scalar(out=xr, in0=x_t, scalar1=mean,
                                                scalar2=rstd,
                                                op0=OP.subtract, op1=OP.mult)
                        nc.gpsimd.tensor_mul(xr, xr, g1b)
                        nc.gpsimd.tensor_add(xn_sh[:, i, :], xr, b1b)

                qkv_macro(0)
                for m in range(8):
                    # stats DMAs/BNStats for the next macro run during this
                    # macro's QKV+scores; the LN finish and the next QKV
                    # land in the mid-cb so they fill the PE while ACT works
                    # through this macro's exp backlog
                    if m < 7:
                        stats_next = ln_stats(x_full, QM * (m + 1))
                        if m == 6:
                            nc.sync.dma_start(out=g1b, in_=g1b_i)
                            nc.sync.dma_start(out=b1b, in_=b1b_i)
                            nc.sync.dma_start(
                                out=dense_sb0,
                                in_=dense_wt[:, 0:QM].rearrange(
                                    "(c p) m -> p c m", p=128))

                        def cb(mm=m, st=stats_next):
                            ln_finish(mm + 1, st)
                            qkv_macro(mm + 1)
                        cb_ = cb
                    else:
                        shard_stats = ln_stats(x_shard, 0)
                        cb_ = lambda sst=shard_stats: shard_finish(sst)
                    attention_macro(m // 4, m % 4, mid_cb=cb_)
                    if m == 3:
                        nc.gpsimd.collective_compute(
                            "AllToAll", mybir.AluOpType.bypass,
                            replica_groups=[list(range(NCORES))],
                            ins=[a2a_in[0].opt()], outs=[a2a_out[0].opt()],
                        )
                        # Pool-queue DMA: queues right behind the collective,
                        # so it neither blocks the SP queue nor adds latency
                        nc.gpsimd.dma_start(out=ctxT[:, :, 0:256],
                                            in_=resh(a2a_out[0]))

                nc.gpsimd.collective_compute(
                    "AllToAll", mybir.AluOpType.bypass,
                    replica_groups=[list(range(NCORES))],
                    ins=[a2a_in[1].opt()], outs=[a2a_out[1].opt()],
                )
                rout1 = resh(a2a_out[1])
                nc.gpsimd.dma_start(out=ctxT[:, 0:4, 256:512],
                                    in_=rout1[:, 0:4, :])
                nc.gpsimd.dma_start(out=ctxT[:, 4:8, 256:512],
                                    in_=rout1[:, 4:8, :])

            # ---------------- dense, LN2, FFN --------------
            with ExitStack() as es2:
                P = lambda *a, **k: es2.enter_context(tc.tile_pool(*a, **k))
                h_pool = P(name="hh", bufs=1)
                st2_pool = P(name="st2", bufs=2)
                hnT_pool = P(name="hnT", bufs=1)
                g1_pool = P(name="g1sb", bufs=1)
                fc_pool = P(name="fcst", bufs=2)
                prj_pool = P(name="prst", bufs=6)
                psd = P(name="psd", bufs=2, space="PSUM")
                pse = P(name="pse", bufs=2, space="PSUM")
                out_pool = P(name="outsb", bufs=1)
                c2_pool = P(name="c2", bufs=1)

                g2b = c2_pool.tile([128, D], dt.float32)
                b2b = c2_pool.tile([128, D], dt.float32)
                fcb = c2_pool.tile([128, 32], dt.float32)
                dense_sb1 = c2_pool.tile([128, 8, QM], dt.bfloat16)
                nc.sync.dma_start(
                    out=dense_sb1,
                    in_=dense_wt[:, QM:D].rearrange("(c p) m -> p c m",
                                                    p=128))
                nc.sync.dma_start(out=g2b, in_=g2b_i)
                nc.sync.dma_start(out=b2b, in_=b2b_i)
                nc.sync.dma_start(out=fcb, in_=fcb_i)

                # h_t is overwritten in place by the (gamma,beta)-applied
                # hn after LN2 reads it (saves 16KB/partition)
                h_t = h_pool.tile([128, 4, D], dt.float32)
                hn_true = h_t
                hnT = hnT_pool.tile([128, 8, TPC], dt.bfloat16)
                g1 = g1_pool.tile([128, 32, TPC], dt.bfloat16)

                # fc weights for ht 0..11 stay resident so the token-half-0
                # fc can bridge the A2A[1] wait; ht 12..31 stream through a
                # small ring
                NRES = 20
                fcr_pool = P(name="fcr", bufs=1)
                fc_res = [fcr_pool.tile([128, 8, 128], dt.bfloat16,
                                        name=f"fcr{ht}")
                          for ht in range(NRES)]
                for ht in range(NRES):
                    nc.sync.dma_start(out=fc_res[ht], in_=fc_wt[ht])

                # proj weights stream in 8 chunks of 8 k-tiles (ring of 7),
                # interleaved with the fc weight stream
                pw = [None] * 8

                def load_pw_chunk(c):
                    pw[c] = prj_pool.tile([128, 8, QM], dt.bfloat16,
                                          tag="pw", name=f"pw{c}")
                    g, dh = c % 4, c // 4
                    nc.sync.dma_start(
                        out=pw[c],
                        in_=proj_wt[1024 * g:1024 * (g + 1),
                                    512 * dh:512 * (dh + 1)]
                        .rearrange("(c p) m -> p c m", p=128))

                def dense_half(hh):
                    """dense + LN2 + hnT for token half hh (2 tiles)."""
                    for ts in (2 * hh, 2 * hh + 1):
                        tsl = slice(128 * ts, 128 * (ts + 1))
                        for dh, dw in ((0, dense_sb0), (1, dense_sb1)):
                            dsl = slice(512 * dh, 512 * (dh + 1))
                            ps = psd.tile([128, QM], dt.float32, tag="dn")
                            for kc in range(8):
                                nc.tensor.matmul(ps, ctxT[:, kc, tsl],
                                                 dw[:, kc, :],
                                                 start=(kc == 0),
                                                 stop=(kc == 7))
                            nc.vector.tensor_add(h_t[:, ts, dsl], ps,
                                                 xn_sh[:, ts, dsl])
                    for i, t in enumerate((2 * hh, 2 * hh + 1)):
                        mv2 = st2_pool.tile([128, 2], dt.float32, tag="mv2")
                        stats = st2_pool.tile([128, 2, 6], dt.float32,
                                              tag="bnst2")
                        nc.vector.bn_stats(out=stats[:, 0, :],
                                           in_=h_t[:, t, 0:512])
                        nc.vector.bn_stats(out=stats[:, 1, :],
                                           in_=h_t[:, t, 512:1024])
                        nc.vector.bn_aggr(out=mv2, in_=stats)
                        lnv2 = st2_pool.tile([128, 1], dt.float32, tag="lnv2")
                        nc.scalar.activation(out=lnv2, in_=mv2[:, 1:2],
                                             func=AF.Ln, scale=VAR_SCALE)
                        rstd2 = st2_pool.tile([128, 1], dt.float32,
                                              tag="rstd2")
                        nc.scalar.activation(out=rstd2, in_=lnv2,
                                             func=AF.Exp, scale=-0.5)
                        hr = st2_pool.tile([128, D], dt.float32, tag="hr",
                                                bufs=2)
                        nc.vector.tensor_scalar(out=hr, in0=h_t[:, t, :],
                                                scalar1=mv2[:, 0:1],
                                                scalar2=rstd2,
                                                op0=OP.subtract, op1=OP.mult)
                        for half in range(2):
                            # fp32 transpose straight from hr (skips the
                            # bf16 staging copy on the latency path)
                            pt = pse.tile([128, 4, 128], dt.float32,
                                          tag="tr2")
                            for s2 in range(4):
                                kc = 4 * half + s2
                                nc.tensor.transpose(
                                    pt[:, s2, :],
                                    hr[:, 128 * kc:128 * (kc + 1)], ident_f32)
                            dst = hnT[:, 4 * half:4 * half + 4,
                                      128 * t:128 * (t + 1)]
                            nc.scalar.copy(out=dst, in_=pt)
                        nc.vector.tensor_mul(hn_true[:, t, :], hr, g2b)
                        nc.vector.tensor_add(hn_true[:, t, :],
                                             hn_true[:, t, :], b2b)

                def fc_half(ht, w, hh):
                    hsl = slice(256 * hh, 256 * (hh + 1))
                    ps = psd.tile([128, 256], dt.float32, tag="fc")
                    for kc in range(8):
                        nc.tensor.matmul(ps, w[:, kc, :], hnT[:, kc, hsl],
                                         start=(kc == 0), stop=(kc == 7))
                    nc.scalar.activation(out=g1[:, ht, hsl], in_=ps,
                                         func=AF.Gelu,
                                         bias=fcb[:, ht:ht + 1], scale=1.0)

                dense_half(0)              # only needs a2a_out[0]
                for ht in range(NRES):     # bridges the A2A[1] wait
                    fc_half(ht, fc_res[ht], 0)
                for ht in range(NRES, 32):  # ring-streamed rest of the h0 fc
                    fcw = fc_pool.tile([128, 8, 128], dt.bfloat16, tag="fcw")
                    nc.sync.dma_start(out=fcw, in_=fc_wt[ht])
                    fc_half(ht, fcw, 0)
                for c in range(6):         # pw stream follows the fcw loads
                    load_pw_chunk(c)
                dense_half(1)
                for ht in range(NRES):
                    fc_half(ht, fc_res[ht], 1)
                for ht in range(NRES, 32):
                    fcw = fc_pool.tile([128, 8, 128], dt.bfloat16, tag="fcw")
                    nc.sync.dma_start(out=fcw, in_=fc_wt[ht])
                    fc_half(ht, fcw, 1)

                # FFN proj: token-major out; out = hn_true + ff.  j order is
                # rotated per ts so chunks 6/7 (which ring onto chunks 0/1's
                # slots) can land while dh=0 finishes.
                load_pw_chunk(6)
                load_pw_chunk(7)
                for dh in range(2):
                    dsl = slice(512 * dh, 512 * (dh + 1))
                    for ts in range(4):
                        tsl = slice(128 * ts, 128 * (ts + 1))
                        ps = pse.tile([128, QM], dt.float32, tag="pj")
                        order = [(8 * ts + k) % 32 for k in range(32)]
                        for j_idx, j in enumerate(order):
                            nc.tensor.matmul(
                                ps, g1[:, j, tsl],
                                pw[4 * dh + j // 8][:, j % 8, :],
                                start=(j_idx == 0), stop=(j_idx == 31))
                        osb = out_pool.tile([128, QM], dt.float32, tag="osb")
                        nc.vector.tensor_add(osb, ps, hn_true[:, ts, dsl])
                        nc.sync.dma_start(out=out_sh[tsl, dsl], in_=osb)

    nc.compile()
    return nc


def _np_reference(x, mask, wq_w, wq_b, wk_w, wk_b, wv_w, wv_b, dense_w,
                  dense_b, gamma1, beta1, gamma2, beta2, fc_w, proj_w):
    """Pure-numpy fallback for non-causal masks (never hit in practice)."""
    import math
    erf = np.vectorize(math.erf)

    def ln(x, g, b):
        mu = x.mean(-1, keepdims=True)
        sd = x.std(-1, ddof=1, keepdims=True)
        return g * ((x - mu) / (sd + 1e-6)) + b

    x = x.astype(np.float64)
    xn = ln(x, gamma1, beta1)
    q = (xn @ wq_w.T + wq_b).reshape(B, S, H, DEP).transpose(0, 2, 1, 3)
    k = (xn @ wk_w.T + wk_b).reshape(B, S, H, DEP).transpose(0, 2, 1, 3)
    v = (xn @ wv_w.T + wv_b).reshape(B, S, H, DEP).transpose(0, 2, 1, 3)
    sc = np.einsum("bhqd,bhkd->bhqk", q, k) / np.sqrt(DEP) + mask * -1e9
    sc = sc - sc.max(-1, keepdims=True)
    e = np.exp(sc)
    a = e / e.sum(-1, keepdims=True)
    ctx = np.einsum("bhqk,bhkd->bhqd", a, v).transpose(0, 2, 1, 3).reshape(
        B, S, D)
    h = xn + ctx @ dense_w.T + dense_b
    hn = ln(h, gamma2, beta2)
    t = hn @ fc_w.T
    g = 0.5 * t * (1.0 + erf(t / np.sqrt(2.0)))
    return (hn + g @ proj_w.T).astype(np.float32)


def _prep_in_maps(inputs):
    x = np.asarray(inputs["x"], np.float32)
    bf16 = ml_dtypes.bfloat16
    g1 = np.asarray(inputs["gamma1"], np.float32)
    b1 = np.asarray(inputs["beta1"], np.float32)
    g2 = np.asarray(inputs["gamma2"], np.float32)
    b2 = np.asarray(inputs["beta2"], np.float32)
    dense_w = np.asarray(inputs["dense_w"], np.float32)
    dense_b = np.asarray(inputs["dense_b"], np.float32)
    fc_w = np.asarray(inputs["fc_w"], np.float32)
    proj_w = np.asarray(inputs["proj_w"], np.float32)

    xf = x.reshape(NT, D).astype(bf16)
    shard_rows = []
    for c in range(NCORES):
        base = 512 * (c // 2) + 256 * (c % 2)
        shard_rows.append(np.concatenate(
            [base + np.arange(256), 2048 + base + np.arange(256)]))
    bcast = lambda v: np.ascontiguousarray(
        np.broadcast_to(v.astype(np.float32), (128, D)))

    # causal triangle block (same for every diagonal sub-block), [k, q]
    md = np.zeros((128, 128), np.float32)
    kk = np.arange(128)[:, None]
    qq = np.arange(128)[None, :]
    md[kk > qq] = NEG

    fc_eff = fc_w * g2[None, :]
    fcb = fc_w @ b2
    in_maps = []
    for c in range(NCORES):
        rows = slice(128 * c, 128 * (c + 1))
        im = {
            "x_full": xf,
            "x_shard": np.ascontiguousarray(xf[shard_rows[c]]),
            "g1b": bcast(g1), "b1b": bcast(b1 + dense_b),
            "g2b": bcast(g2), "b2b": bcast(b2),
            "dense_wt": dense_w.T.astype(bf16),
            "fc_wt": np.ascontiguousarray(
                fc_eff.T.reshape(8, 128, 32, 128).transpose(
                    2, 1, 0, 3)).astype(bf16),
            "fcb": np.ascontiguousarray(fcb.reshape(32, 128).T),
            "proj_wt": proj_w.T.astype(bf16),
            "mask_tri": md.astype(bf16),
        }
        for nm, w, bias in (("q", np.asarray(inputs["wq_w"], np.float32),
                             np.asarray(inputs["wq_b"], np.float32)),
                            ("k", np.asarray(inputs["wk_w"], np.float32),
                             np.asarray(inputs["wk_b"], np.float32)),
                            ("v", np.asarray(inputs["wv_w"], np.float32),
                             np.asarray(inputs["wv_b"], np.float32))):
            wslice = w[rows]                     # [128, D]
            im[f"w{nm}t"] = np.ascontiguousarray(
                (wslice * g1[None, :]).T.reshape(8, 128, 128).transpose(
                    1, 0, 2)).astype(bf16)
            im[f"{nm}b"] = (bias[rows] + wslice @ b1).reshape(128, 1)
        in_maps.append(im)
    return in_maps, shard_rows


def kernel(**inputs):
    mask = np.asarray(inputs["mask"], np.float32)
    causal = np.array_equal(mask, np.triu(np.ones((S, S), np.float32), k=1))
    if not causal:
        return _np_reference(**{k: np.asarray(v, np.float64 if
                                              np.asarray(v).dtype != np.int32
                                              else np.int32)
                                for k, v in inputs.items()}).reshape(B, S, D)

    if "nc" not in _cache:
        _cache["nc"] = _build_program()
    nc = _cache["nc"]

    in_maps, shard_rows = _prep_in_maps(inputs)
    global _last_in_maps
    _last_in_maps = in_maps
    from concourse import bass_utils
    res = bass_utils.run_bass_kernel_spmd(nc, in_maps,
                                          core_ids=list(range(NCORES)))
    out = np.empty((NT, D), np.float32)
    for c in range(NCORES):
        out[shard_rows[c]] = res.results[c]["out_shard"]
    return out.reshape(B, S, D)


# revision 90
# speedup vs baseline: 1.0070x; 1.0070x over previous
"""Trainium2 Bass kernel for a decoder block (LN -> MHA -> LN -> FFN).

Sharding: heads across the 8 cores for attention (2 heads/core), tokens
across cores for dense/LN2/FFN (512 tokens/core), connected by an
AllToAll of the softmax-normalized ctx in bf16 — split into one
collective per batch so the first overlaps batch-1 attention.

All matmuls run in bf16 with fp32 PSUM accumulation; layernorm stats,
softmax input, and residuals stay fp32.  gamma1/beta1 are folded into
the QKV weights/biases on the host, gamma2/beta2 into fc_w and a fc
output bias; the true (gamma,beta)-applied xn / hn needed for the
residual connections are computed on-chip for the core's own token
shard only.

Engine budget notes:
 - x ships as bf16 (halves DMA, enables the 4x DVE tensor_scalar mode
   for the LN normalize); weight tensors are host-relayout'd so every
   DMA row is >= 512B (avoids the small-descriptor 2x penalty).
 - rstd = exp(-0.5*ln(c*var)): keeps every phase-1 activation in the
   natural_log_exp_and_others table set (one pre-placed load, no
   mid-kernel ACT table switches; Gelu loads once in phase 2).
 - causal mask lands in PSUM via an identity-matmul prefill of the
   shared triangle block; the diagonal scores matmul accumulates onto
   it (no element-wise mask adds).
 - softmax exp runs once per k-tile over both heads' scores; the
   next macro's LN-finish + QKV are emitted between scores and ctx so
   the PE stays busy while ACT drains the exp backlog.
 - softmax denominator: reciprocal row is broadcast to rows 64:128 of
   the ctx PSUM bank by a ones-matmul, staged through SBUF for the
   normalize multiply (HW allows only one PSUM operand per DVE op).
 - the batch-0 A2A is issued mid-phase-1; the batch-1 A2A is bridged
   by dense/LN2/fc work on the batch-0 token half (fc weights for the
   first NRES rows stay resident so no DMA sits on that path).
 - proj weights stream in 8 k-tile chunks through a 6-slot ring,
   interleaved with the fc weight stream; the proj j-order is rotated
   per token tile so the ring reuse lands just in time.
"""

from contextlib import ExitStack

import numpy as np
import ml_dtypes

B, S, D = 2, 2048, 1024
H, DEP = 16, 64
NT = B * S            # 4096 flattened tokens
NCORES = 8
HPC = H // NCORES     # 2 heads per core
TPC = NT // NCORES    # 512 tokens per core
QM = 512              # q-macro / token-macro size
KT = 128              # k-tile size
NEG = -1.0e9

_cache = {}


def _build_program():
    import concourse.bacc as bacc
    import concourse.tile as tile
    import concourse.mybir as mybir
    from concourse.masks import make_identity
    from concourse.hw_specs import get_activation_tables

    dt = mybir.dt
    AF = mybir.ActivationFunctionType
    OP = mybir.AluOpType

    nc = bacc.Bacc("TRN2", target_bir_lowering=False, debug=False,
                   num_devices=NCORES)

    def din(name, shape, dtype=dt.float32):
        return nc.dram_tensor(name, shape, dtype, kind="ExternalInput").ap()

    x_full = din("x_full", [NT, D], dt.bfloat16)
    x_shard = din("x_shard", [TPC, D], dt.bfloat16)
    wqt = din("wqt", [128, 8, 128], dt.bfloat16)
    wkt = din("wkt", [128, 8, 128], dt.bfloat16)
    wvt = din("wvt", [128, 8, 128], dt.bfloat16)
    qb_i = din("qb", [128, 1])
    kb_i = din("kb", [128, 1])
    vb_i = din("vb", [128, 1])
    g1b_i = din("g1b", [128, D])
    b1b_i = din("b1b", [128, D])
    g2b_i = din("g2b", [128, D])
    b2b_i = din("b2b", [128, D])
    dense_wt = din("dense_wt", [D, D], dt.bfloat16)
    fc_wt = din("fc_wt", [32, 128, 8, 128], dt.bfloat16)
    fcb_i = din("fcb", [128, 32])
    proj_wt = din("proj_wt", [4 * D, D], dt.bfloat16)
    mask_i = din("mask_tri", [128, 128], dt.bfloat16)
    out_sh = nc.dram_tensor("out_shard", [TPC, D], dt.float32,
                            kind="ExternalOutput").ap()

    VAR_SCALE = float(D) / float(D - 1)   # ddof=1 correction

    set_names = [t[0] for t in get_activation_tables(nc.m.arch).items()]
    NLE_SET = set_names.index("natural_log_exp_and_others")

    with tile.TileContext(nc) as tc:
        with ExitStack() as es0:
            consts = es0.enter_context(tc.tile_pool(name="consts", bufs=1))
            dram = es0.enter_context(
                tc.tile_pool(name="dram", bufs=1, space="DRAM"))
            # pre-place the ln+exp table so phase 1 never switches sets
            nc.scalar.add_instruction(mybir.InstLoadActFuncSet(
                name=nc.get_next_instruction_name(),
                act_func_set_id=NLE_SET, ins=[], outs=[]))
            ident_bf = consts.tile([128, 128], dt.bfloat16)
            make_identity(nc, ident_bf)
            ones_bf = consts.tile([1, 64], dt.bfloat16)
            nc.vector.memset(ones_bf, 1.0)
            ident_f32 = consts.tile([128, 128], dt.float32)
            make_identity(nc, ident_f32)
            qb = consts.tile([128, 1], dt.float32)
            kb = consts.tile([128, 1], dt.float32)
            vb = consts.tile([128, 1], dt.float32)
            mask_tri = consts.tile([128, 128], dt.bfloat16)

            xnsh_pool = es0.enter_context(tc.tile_pool(name="xnsh", bufs=1))
            xn_sh = xnsh_pool.tile([128, 4, D], dt.float32)
            p2pre = es0.enter_context(tc.tile_pool(name="p2pre", bufs=1))
            ctxT = p2pre.tile([128, 8, TPC], dt.bfloat16)
            dense_sb0 = p2pre.tile([128, 8, QM], dt.bfloat16)

            # batch-mixed shards: core c owns 256 tokens of each batch
            # (batch b, macro c//2, half c%2), so both A2As carry only
            # real data and land in disjoint token ranges of ctxT.
            a2a_in = [dram.tile([NCORES, HPC, DEP, QM // 2], dt.bfloat16,
                                name=f"a2a_in{bb}") for bb in range(2)]
            a2a_out = [dram.tile([NCORES, HPC, DEP, QM // 2], dt.bfloat16,
                                 name=f"a2a_out{bb}") for bb in range(2)]
            resh = lambda t: t.rearrange("c h d q -> (c h d) q").rearrange(
                "(k p) q -> p k q", p=128)

            # ------- LN1 + QKV interleaved, then attention per batch -------
            with ExitStack() as es1:
                P = lambda *a, **k: es1.enter_context(tc.tile_pool(*a, **k))
                xt_pool = P(name="xt", bufs=2)
                st_pool = P(name="stats", bufs=3)
                xnT_pool = P(name="xnT", bufs=1)
                qkT_pool = P(name="qkT", bufs=1)
                v_pool = P(name="vtok", bufs=1)
                wq_pool = P(name="wq", bufs=1)
                ps_tr = P(name="ps_tr", bufs=1, space="PSUM")
                ps_sc = P(name="ps_sc", bufs=2, space="PSUM")
                ps_qk = P(name="ps_qk", bufs=1, space="PSUM")
                pr_pool = P(name="probs", bufs=16)
                psctx = P(name="psctx", bufs=2, space="PSUM")
                rb_pool = P(name="rbp", bufs=2)

                xn_T = xnT_pool.tile([128, 8, NT], dt.bfloat16)
                q_T = qkT_pool.tile([128, NT], dt.bfloat16)
                k_T = qkT_pool.tile([128, NT], dt.bfloat16)
                vtok = v_pool.tile([128, 32, 130], dt.bfloat16)
                nc.vector.memset(vtok[:, :, 64:65], 1.0)
                nc.vector.memset(vtok[:, :, 129:130], 1.0)

                ps_trT = ps_tr.tile([128, 8, 128], dt.bfloat16)
                wq_sb = wq_pool.tile([128, 8, 128], dt.bfloat16)
                wk_sb = wq_pool.tile([128, 8, 128], dt.bfloat16)
                wv_sb = wq_pool.tile([128, 8, 128], dt.bfloat16)
                g1b = wq_pool.tile([128, D], dt.float32)
                b1b = wq_pool.tile([128, D], dt.float32)

                def ln_stats(src, base_row):
                    """Stats for 4 consecutive 128-row tiles; per-tile rstd
                    via exp(-0.5*ln(c*var)) (stays in the ln/exp table set).
                    Returns list of (x_t, mean_ap, rstd_ap)."""
                    mv4 = st_pool.tile([128, 4, 2], dt.float32, tag="mv4")
                    rstd4 = st_pool.tile([128, 4], dt.float32, tag="rstd4")
                    xts = []
                    for i in range(4):
                        x_t = xt_pool.tile([128, D], dt.bfloat16, tag="xt",
                                           bufs=5)
                        r0 = base_row + 128 * i
                        nc.sync.dma_start(out=x_t, in_=src[r0:r0 + 128, :])
                        stats = st_pool.tile([128, 2, 6], dt.float32,
                                             tag="bnst")
                        nc.vector.bn_stats(out=stats[:, 0, :],
                                           in_=x_t[:, 0:512])
                        nc.vector.bn_stats(out=stats[:, 1, :],
                                           in_=x_t[:, 512:1024])
                        nc.vector.bn_aggr(out=mv4[:, i, :], in_=stats)
                        lnv = st_pool.tile([128, 1], dt.float32, tag="lnv")
                        nc.scalar.activation(out=lnv, in_=mv4[:, i, 1:2],
                                             func=AF.Ln, scale=VAR_SCALE)
                        nc.scalar.activation(out=rstd4[:, i:i + 1], in_=lnv,
                                             func=AF.Exp, scale=-0.5)
                        xts.append(x_t)
                    return [(xts[i], mv4[:, i, 0:1], rstd4[:, i:i + 1])
                            for i in range(4)]

                def ln_finish(m, stats, on_act=False, tile0=0):
                    """normalize + transpose + copy into xn_T for macro m.
                    Copies go to ACT in windows where DVE is the critical
                    engine (small attention macros), else DVE."""
                    for i, (x_t, mean, rstd) in enumerate(stats):
                        t = 4 * m + tile0 + i
                        xnb = xt_pool.tile([128, D], dt.bfloat16, tag="xnb",
                                           bufs=2)
                        nc.vector.tensor_scalar(out=xnb, in0=x_t, scalar1=mean,
                                                scalar2=rstd, op0=OP.subtract,
                                                op1=OP.mult)
                        for half in range(2):
                            ps = ps_trT[:, 4 * half:4 * half + 4, :]
                            for s2 in range(4):
                                kc = 4 * half + s2
                                nc.tensor.transpose(
                                    ps[:, s2, :],
                                    xnb[:, 128 * kc:128 * (kc + 1)], ident_bf)
                            dst = xn_T[:, 4 * half:4 * half + 4,
                                       128 * t:128 * (t + 1)]
                            if on_act:
                                nc.scalar.copy(out=dst, in_=ps)
                            else:
                                nc.vector.tensor_copy(out=dst, in_=ps)

                def qkv_macro(m):
                    # bias adds go to DVE when this emission runs inside a
                    # long-exp window (ACT queue is the pacer there)
                    dve_adds = m in (7,)
                    tok = slice(QM * m, QM * (m + 1))
                    for w_sb, bias, dst in ((wq_sb, qb, q_T), (wk_sb, kb, k_T)):
                        ps = ps_qk.tile([128, QM], dt.float32, tag="qk")
                        for kc in range(8):
                            nc.tensor.matmul(ps, w_sb[:, kc, :],
                                             xn_T[:, kc, tok],
                                             start=(kc == 0), stop=(kc == 7))
                        if dve_adds:
                            nc.vector.tensor_scalar_add(dst[:, tok], ps, bias)
                        else:
                            nc.scalar.add(out=dst[:, tok], in_=ps, add=bias)
                    ps = ps_qk.tile([128, QM], dt.float32, tag="qk")
                    for kc in range(8):
                        nc.tensor.matmul(ps, wv_sb[:, kc, :], xn_T[:, kc, tok],
                                         start=(kc == 0), stop=(kc == 7))
                    vst = rb_pool.tile([128, QM], dt.bfloat16, tag="vst",
                                       bufs=2)
                    if dve_adds:
                        nc.vector.tensor_scalar_add(vst, ps, vb)
                    else:
                        nc.scalar.add(out=vst, in_=ps, add=vb)
                    for half in range(2):
                        pt = ps_trT[:, 2 * half:2 * half + 2, :]
                        for s2 in range(2):
                            s = 2 * half + s2
                            nc.tensor.transpose(
                                pt[:, s2, :], vst[:, 128 * s:128 * (s + 1)],
                                ident_bf)
                        for s2 in range(2):
                            kt_idx = 4 * m + 2 * half + s2
                            dst = vtok[:, kt_idx, 0:130].rearrange(
                                "p (two h) -> p two h", two=2)[:, :, 0:64]
                            src = pt[:, s2, :].rearrange(
                                "p (two h) -> p two h", two=2)
                            nc.vector.tensor_copy(out=dst, in_=src)

                def attention_macro(b, mm, mid_cb=None):
                    q0 = 2048 * b + QM * mm
                    nkt = 4 * mm + 4
                    probs = []
                    for j in range(nkt):
                        rel = j - 4 * mm
                        lo = 128 * rel if rel > 0 else 0
                        ks = slice(2048 * b + KT * j,
                                   2048 * b + KT * (j + 1))
                        ps = ps_sc.tile([128, 2, QM], dt.float32, tag="sc")
                        pb = pr_pool.tile([128, 2, QM], dt.bfloat16, tag="pr")
                        for h in range(2):
                            hp = slice(64 * h, 64 * (h + 1))
                            if rel >= 0:
                                hi = lo + 128
                                nc.tensor.matmul(
                                    ps[:, h, lo:hi], ident_bf, mask_tri,
                                    start=True, stop=False)
                                nc.tensor.matmul(
                                    ps[:, h, lo:hi], k_T[hp, ks],
                                    q_T[hp, q0 + lo:q0 + hi],
                                    start=False, stop=True)
                                if hi < QM:
                                    nc.tensor.matmul(
                                        ps[:, h, hi:QM], k_T[hp, ks],
                                        q_T[hp, q0 + hi:q0 + QM],
                                        start=True, stop=True)
                            else:
                                nc.tensor.matmul(
                                    ps[:, h, :], k_T[hp, ks],
                                    q_T[hp, q0:q0 + QM],
                                    start=True, stop=True)
                        nc.scalar.activation(out=pb[:, :, lo:QM],
                                             in_=ps[:, :, lo:QM],
                                             func=AF.Exp, scale=0.125)
                        probs.append((pb, lo))
                    if mid_cb is not None:
                        # emitted between scores/exp and ctx: the next
                        # macro's LN work lands on DVE/PE while ACT chews
                        # through this macro's exps
                        mid_cb()
                    pcs = []
                    for h in range(2):
                        pc = psctx.tile([128, QM], dt.float32, tag="ctx")
                        for j in range(nkt):
                            pb, lo = probs[j]
                            nc.tensor.matmul(
                                pc[0:65, lo:QM],
                                vtok[:, 16 * b + j, 65 * h:65 * (h + 1)],
                                pb[:, h, lo:QM],
                                start=(j == 0), stop=(j == nkt - 1))
                        pcs.append(pc)

                    def fin_bc(h):
                        # denominator: reciprocal row broadcast to rows
                        # 64:128 of the ctx PSUM bank via a ones-matmul,
                        # staged through SBUF (HW allows only one PSUM
                        # operand on the DVE multiply)
                        pc = pcs[h]
                        r32 = rb_pool.tile([1, QM], dt.float32, tag="r32")
                        nc.vector.reciprocal(out=r32, in_=pc[64:65, :])
                        rbf = rb_pool.tile([1, QM], dt.bfloat16, tag="rbf")
                        nc.vector.tensor_copy(out=rbf, in_=r32)
                        nc.tensor.matmul(pc[64:128, :], ones_bf, rbf,
                                         start=True, stop=True)
                        rb = rb_pool.tile([64, QM], dt.bfloat16, tag="rb",
                                          name=f"rb{h}")
                        nc.scalar.copy(out=rb, in_=pc[64:128, :])
                        return rb

                    def fin_csb(h, rb):
                        csb = rb_pool.tile([64, QM], dt.bfloat16, tag="csb")
                        nc.vector.tensor_tensor(out=csb, in0=pcs[h][0:64, :],
                                                in1=rb, op=OP.mult)
                        for hf in range(2):
                            nc.sync.dma_start(
                                out=a2a_in[b][2 * mm + hf, h],
                                in_=csb[:, 256 * hf:256 * (hf + 1)])

                    rb0 = fin_bc(0)
                    rb1 = fin_bc(1)
                    fin_csb(0, rb0)
                    fin_csb(1, rb1)

                # first macro's x tiles go down the DMA queue first, then
                # the weights/consts needed a few microseconds later; the
                # compute chain is interleaved per tile so tile 0's
                # normalize+transpose isn't queued behind tiles 1-3's stats
                mv4p = st_pool.tile([128, 4, 2], dt.float32, tag="mv4")
                rstd4p = st_pool.tile([128, 4], dt.float32, tag="rstd4")
                xts_p = []
                for i in range(4):
                    x_t = xt_pool.tile([128, D], dt.bfloat16, tag="xt",
                                       bufs=5)
                    nc.sync.dma_start(out=x_t,
                                      in_=x_full[128 * i:128 * (i + 1), :])
                    xts_p.append(x_t)
                nc.sync.dma_start(out=wq_sb, in_=wqt)
                nc.sync.dma_start(out=qb, in_=qb_i)
                nc.sync.dma_start(out=wk_sb, in_=wkt)
                nc.sync.dma_start(out=kb, in_=kb_i)
                nc.sync.dma_start(out=wv_sb, in_=wvt)
                nc.sync.dma_start(out=vb, in_=vb_i)
                nc.sync.dma_start(out=mask_tri, in_=mask_i)
                for i in range(4):
                    stats = st_pool.tile([128, 2, 6], dt.float32, tag="bnst")
                    nc.vector.bn_stats(out=stats[:, 0, :],
                                       in_=xts_p[i][:, 0:512])
                    nc.vector.bn_stats(out=stats[:, 1, :],
                                       in_=xts_p[i][:, 512:1024])
                    nc.vector.bn_aggr(out=mv4p[:, i, :], in_=stats)
                    lnv = st_pool.tile([128, 1], dt.float32, tag="lnv")
                    nc.scalar.activation(out=lnv, in_=mv4p[:, i, 1:2],
                                         func=AF.Ln, scale=VAR_SCALE)
                    nc.scalar.activation(out=rstd4p[:, i:i + 1], in_=lnv,
                                         func=AF.Exp, scale=-0.5)
                    ln_finish(0, [(xts_p[i], mv4p[:, i, 0:1],
                                   rstd4p[:, i:i + 1])], tile0=i,
                              on_act=True)

                def shard_finish(stats):
                    for i, (x_t, mean, rstd) in enumerate(stats):
                        xr = xt_pool.tile([128, D], dt.float32, tag="xr",
                                          bufs=2)
                        nc.vector.tensor_scalar(out=xr, in0=x_t, scalar1=mean,
                                                scalar2=rstd,
                                                op0=OP.subtract, op1=OP.mult)
                        nc.gpsimd.tensor_mul(xr, xr, g1b)
                        nc.gpsimd.tensor_add(xn_sh[:, i, :], xr, b1b)

                qkv_macro(0)
                for m in range(8):
                    # stats DMAs/BNStats for the next macro run during this
                    # macro's QKV+scores; the LN finish and the next QKV
                    # land in the mid-cb so they fill the PE while ACT works
                    # through this macro's exp backlog
                    if m < 7:
                        stats_next = ln_stats(x_full, QM * (m + 1))
                        if m == 6:
                            nc.sync.dma_start(out=g1b, in_=g1b_i)
                            nc.sync.dma_start(out=b1b, in_=b1b_i)
                            nc.sync.dma_start(
                                out=dense_sb0,
                                in_=dense_wt[:, 0:QM].rearrange(
                                    "(c p) m -> p c m", p=128))

                        def cb(mm=m, st=stats_next):
                            ln_finish(mm + 1, st)
                            qkv_macro(mm + 1)
                        cb_ = cb
                    else:
                        shard_stats = ln_stats(x_shard, 0)
                        cb_ = lambda sst=shard_stats: shard_finish(sst)
                    attention_macro(m // 4, m % 4, mid_cb=cb_)
                    if m == 3:
                        nc.gpsimd.collective_compute(
                            "AllToAll", mybir.AluOpType.bypass,
                            replica_groups=[list(range(NCORES))],
                            ins=[a2a_in[0].opt()], outs=[a2a_out[0].opt()],
                        )
                        # Pool-queue DMA: queues right behind the collective,
                        # so it neither blocks the SP queue nor adds latency
                        nc.gpsimd.dma_start(out=ctxT[:, :, 0:256],
                                            in_=resh(a2a_out[0]))

                nc.gpsimd.collective_compute(
                    "AllToAll", mybir.AluOpType.bypass,
                    replica_groups=[list(range(NCORES))],
                    ins=[a2a_in[1].opt()], outs=[a2a_out[1].opt()],
                )
                rout1 = resh(a2a_out[1])
                nc.gpsimd.dma_start(out=ctxT[:, 0:4, 256:512],
                                    in_=rout1[:, 0:4, :])
                nc.gpsimd.dma_start(out=ctxT[:, 4:8, 256:512],
                                    in_=rout1[:, 4:8, :])

            # ---------------- dense, LN2, FFN --------------
            with ExitStack() as es2:
                P = lambda *a, **k: es2.enter_context(tc.tile_pool(*a, **k))
                h_pool = P(name="hh", bufs=1)
                st2_pool = P(name="st2", bufs=2)
                hnT_pool = P(name="hnT", bufs=1)
                g1_pool = P(name="g1sb", bufs=1)
                fc_pool = P(name="fcst", bufs=2)
                prj_pool = P(name="prst", bufs=6)
                psd = P(name="psd", bufs=2, space="PSUM")
                pse = P(name="pse", bufs=2, space="PSUM")
                out_pool = P(name="outsb", bufs=1)
                c2_pool = P(name="c2", bufs=1)

                g2b = c2_pool.tile([128, D], dt.float32)
                b2b = c2_pool.tile([128, D], dt.float32)
                fcb = c2_pool.tile([128, 32], dt.float32)
                dense_sb1 = c2_pool.tile([128, 8, QM], dt.bfloat16)
                nc.sync.dma_start(
                    out=dense_sb1,
                    in_=dense_wt[:, QM:D].rearrange("(c p) m -> p c m",
                                                    p=128))
                nc.sync.dma_start(out=g2b, in_=g2b_i)
                nc.sync.dma_start(out=b2b, in_=b2b_i)
                nc.sync.dma_start(out=fcb, in_=fcb_i)

                # h_t is overwritten in place by the (gamma,beta)-applied
                # hn after LN2 reads it (saves 16KB/partition)
                h_t = h_pool.tile([128, 4, D], dt.float32)
                hn_true = h_t
                hnT = hnT_pool.tile([128, 8, TPC], dt.bfloat16)
                g1 = g1_pool.tile([128, 32, TPC], dt.bfloat16)

                # fc weights for ht 0..11 stay resident so the token-half-0
                # fc can bridge the A2A[1] wait; ht 12..31 stream through a
                # small ring
                NRES = 20
                fcr_pool = P(name="fcr", bufs=1)
                fc_res = [fcr_pool.tile([128, 8, 128], dt.bfloat16,
                                        name=f"fcr{ht}")
                          for ht in range(NRES)]
                for ht in range(NRES):
                    nc.sync.dma_start(out=fc_res[ht], in_=fc_wt[ht])

                # proj weights stream in 8 chunks of 8 k-tiles (ring of 7),
                # interleaved with the fc weight stream
                pw = [None] * 8

                def load_pw_chunk(c):
                    pw[c] = prj_pool.tile([128, 8, QM], dt.bfloat16,
                                          tag="pw", name=f"pw{c}")
                    g, dh = c % 4, c // 4
                    nc.sync.dma_start(
                        out=pw[c],
                        in_=proj_wt[1024 * g:1024 * (g + 1),
                                    512 * dh:512 * (dh + 1)]
                        .rearrange("(c p) m -> p c m", p=128))

                def dense_half(hh):
                    """dense + LN2 + hnT for token half hh (2 tiles)."""
                    for ts in (2 * hh, 2 * hh + 1):
                        tsl = slice(128 * ts, 128 * (ts + 1))
                        for dh, dw in ((0, dense_sb0), (1, dense_sb1)):
                            dsl = slice(512 * dh, 512 * (dh + 1))
                            ps = psd.tile([128, QM], dt.float32, tag="dn")
                            for kc in range(8):
                                nc.tensor.matmul(ps, ctxT[:, kc, tsl],
                                                 dw[:, kc, :],
                                                 start=(kc == 0),
                                                 stop=(kc == 7))
                            nc.vector.tensor_add(h_t[:, ts, dsl], ps,
                                                 xn_sh[:, ts, dsl])
                    for i, t in enumerate((2 * hh, 2 * hh + 1)):
                        mv2 = st2_pool.tile([128, 2], dt.float32, tag="mv2")
                        stats = st2_pool.tile([128, 2, 6], dt.float32,
                                              tag="bnst2")
                        nc.vector.bn_stats(out=stats[:, 0, :],
                                           in_=h_t[:, t, 0:512])
                        nc.vector.bn_stats(out=stats[:, 1, :],
                                           in_=h_t[:, t, 512:1024])
                        nc.vector.bn_aggr(out=mv2, in_=stats)
                        lnv2 = st2_pool.tile([128, 1], dt.float32, tag="lnv2")
                        nc.scalar.activation(out=lnv2, in_=mv2[:, 1:2],
                                             func=AF.Ln, scale=VAR_SCALE)
                        rstd2 = st2_pool.tile([128, 1], dt.float32,
                                              tag="rstd2")
                        nc.scalar.activation(out=rstd2, in_=lnv2,
                                             func=AF.Exp, scale=-0.5)
                        hr = st2_pool.tile([128, D], dt.float32, tag="hr",
                                                bufs=2)
                        nc.vector.tensor_scalar(out=hr, in0=h_t[:, t, :],
                                                scalar1=mv2[:, 0:1],
                                                scalar2=rstd2,
                                                op0=OP.subtract, op1=OP.mult)
                        for half in range(2):
                            # fp32 transpose straight from hr (skips the
                            # bf16 staging copy on the latency path)
                            pt = pse.tile([128, 4, 128], dt.float32,
                                          tag="tr2")
                            for s2 in range(4):
                                kc = 4 * half + s2
                                nc.tensor.transpose(
                                    pt[:, s2, :],
                                    hr[:, 128 * kc:128 * (kc + 1)], ident_f32)
                            dst = hnT[:, 4 * half:4 * half + 4,
                                      128 * t:128 * (t + 1)]
                            nc.scalar.copy(out=dst, in_=pt)
                        nc.vector.tensor_mul(hn_true[:, t, :], hr, g2b)
                        nc.vector.tensor_add(hn_true[:, t, :],
                                             hn_true[:, t, :], b2b)

                def fc_half(ht, w, hh):
                    hsl = slice(256 * hh, 256 * (hh + 1))
                    ps = psd.tile([128, 256], dt.float32, tag="fc")
                    for kc in range(8):
                        nc.tensor.matmul(ps, w[:, kc, :], hnT[:, kc, hsl],
                                         start=(kc == 0), stop=(kc == 7))
                    nc.scalar.activation(out=g1[:, ht, hsl], in_=ps,
                                         func=AF.Gelu,
                                         bias=fcb[:, ht:ht + 1], scale=1.0)

                dense_half(0)              # only needs a2a_out[0]
                for ht in range(NRES):     # bridges the A2A[1] wait
                    fc_half(ht, fc_res[ht], 0)
                for ht in range(NRES, 32):  # ring-streamed rest of the h0 fc
                    fcw = fc_pool.tile([128, 8, 128], dt.bfloat16, tag="fcw")
                    nc.sync.dma_start(out=fcw, in_=fc_wt[ht])
                    fc_half(ht, fcw, 0)
                for c in range(6):         # pw stream follows the fcw loads
                    load_pw_chunk(c)
                dense_half(1)
                for ht in range(NRES):
                    fc_half(ht, fc_res[ht], 1)
                for ht in range(NRES, 32):
                    fcw = fc_pool.tile([128, 8, 128], dt.bfloat16, tag="fcw")
                    nc.sync.dma_start(out=fcw, in_=fc_wt[ht])
                    fc_half(ht, fcw, 1)

                # FFN proj: token-major out; out = hn_true + ff.  j order is
                # rotated per ts so chunks 6/7 (which ring onto chunks 0/1's
                # slots) can land while dh=0 finishes.
                load_pw_chunk(6)
                load_pw_chunk(7)
                for dh in range(2):
                    dsl = slice(512 * dh, 512 * (dh + 1))
                    for ts in range(4):
                        tsl = slice(128 * ts, 128 * (ts + 1))
                        ps = pse.tile([128, QM], dt.float32, tag="pj")
                        order = [(8 * ts + k) % 32 for k in range(32)]
                        for j_idx, j in enumerate(order):
                            nc.tensor.matmul(
                                ps, g1[:, j, tsl],
                                pw[4 * dh + j // 8][:, j % 8, :],
                                start=(j_idx == 0), stop=(j_idx == 31))
                        osb = out_pool.tile([128, QM], dt.float32, tag="osb")
                        nc.vector.tensor_add(osb, ps, hn_true[:, ts, dsl])
                        nc.sync.dma_start(out=out_sh[tsl, dsl], in_=osb)

    nc.compile()
    return nc


def _np_reference(x, mask, wq_w, wq_b, wk_w, wk_b, wv_w, wv_b, dense_w,
                  dense_b, gamma1, beta1, gamma2, beta2, fc_w, proj_w):
    """Pure-numpy fallback for non-causal masks (never hit in practice)."""
    import math
    erf = np.vectorize(math.erf)

    def ln(x, g, b):
        mu = x.mean(-1, keepdims=True)
        sd = x.std(-1, ddof=1, keepdims=True)
        return g * ((x - mu) / (sd + 1e-6)) + b

    x = x.astype(np.float64)
    xn = ln(x, gamma1, beta1)
    q = (xn @ wq_w.T + wq_b).reshape(B, S, H, DEP).transpose(0, 2, 1, 3)
    k = (xn @ wk_w.T + wk_b).reshape(B, S, H, DEP).transpose(0, 2, 1, 3)
    v = (xn @ wv_w.T + wv_b).reshape(B, S, H, DEP).transpose(0, 2, 1, 3)
    sc = np.einsum("bhqd,bhkd->bhqk", q, k) / np.sqrt(DEP) + mask * -1e9
    sc = sc - sc.max(-1, keepdims=True)
    e = np.exp(sc)
    a = e / e.sum(-1, keepdims=True)
    ctx = np.einsum("bhqk,bhkd->bhqd", a, v).transpose(0, 2, 1, 3).reshape(
        B, S, D)
    h = xn + ctx @ dense_w.T + dense_b
    hn = ln(h, gamma2, beta2)
    t = hn @ fc_w.T
    g = 0.5 * t * (1.0 + erf(t / np.sqrt(2.0)))
    return (hn + g @ proj_w.T).astype(np.float32)


def _prep_in_maps(inputs):
    x = np.asarray(inputs["x"], np.float32)
    bf16 = ml_dtypes.bfloat16
    g1 = np.asarray(inputs["gamma1"], np.float32)
    b1 = np.asarray(inputs["beta1"], np.float32)
    g2 = np.asarray(inputs["gamma2"], np.float32)
    b2 = np.asarray(inputs["beta2"], np.float32)
    dense_w = np.asarray(inputs["dense_w"], np.float32)
    dense_b = np.asarray(inputs["dense_b"], np.float32)
    fc_w = np.asarray(inputs["fc_w"], np.float32)
    proj_w = np.asarray(inputs["proj_w"], np.float32)

    xf = x.reshape(NT, D).astype(bf16)
    shard_rows = []
    for c in range(NCORES):
        base = 512 * (c // 2) + 256 * (c % 2)
        shard_rows.append(np.concatenate(
            [base + np.arange(256), 2048 + base + np.arange(256)]))
    bcast = lambda v: np.ascontiguousarray(
        np.broadcast_to(v.astype(np.float32), (128, D)))

    # causal triangle block (same for every diagonal sub-block), [k, q]
    md = np.zeros((128, 128), np.float32)
    kk = np.arange(128)[:, None]
    qq = np.arange(128)[None, :]
    md[kk > qq] = NEG

    fc_eff = fc_w * g2[None, :]
    fcb = fc_w @ b2
    in_maps = []
    for c in range(NCORES):
        rows = slice(128 * c, 128 * (c + 1))
        im = {
            "x_full": xf,
            "x_shard": np.ascontiguousarray(xf[shard_rows[c]]),
            "g1b": bcast(g1), "b1b": bcast(b1 + dense_b),
            "g2b": bcast(g2), "b2b": bcast(b2),
            "dense_wt": dense_w.T.astype(bf16),
            "fc_wt": np.ascontiguousarray(
                fc_eff.T.reshape(8, 128, 32, 128).transpose(
                    2, 1, 0, 3)).astype(bf16),
            "fcb": np.ascontiguousarray(fcb.reshape(32, 128).T),
            "proj_wt": proj_w.T.astype(bf16),
            "mask_tri": md.astype(bf16),
        }
        for nm, w, bias in (("q", np.asarray(inputs["wq_w"], np.float32),
                             np.asarray(inputs["wq_b"], np.float32)),
                            ("k", np.asarray(inputs["wk_w"], np.float32),
                             np.asarray(inputs["wk_b"], np.float32)),
                            ("v", np.asarray(inputs["wv_w"], np.float32),
                             np.asarray(inputs["wv_b"], np.float32))):
            wslice = w[rows]                     # [128, D]
            im[f"w{nm}t"] = np.ascontiguousarray(
                (wslice * g1[None, :]).T.reshape(8, 128, 128).transpose(
                    1, 0, 2)).astype(bf16)
            im[f"{nm}b"] = (bias[rows] + wslice @ b1).reshape(128, 1)
        in_maps.append(im)
    return in_maps, shard_rows


def kernel(**inputs):
    mask = np.asarray(inputs["mask"], np.float32)
    causal = np.array_equal(mask, np.triu(np.ones((S, S), np.float32), k=1))
    if not causal:
        return _np_reference(**{k: np.asarray(v, np.float64 if
                                              np.asarray(v).dtype != np.int32
                                              else np.int32)
                                for k, v in inputs.items()}).reshape(B, S, D)

    if "nc" not in _cache:
        _cache["nc"] = _build_program()
    nc = _cache["nc"]

    in_maps, shard_rows = _prep_in_maps(inputs)
    global _last_in_maps
    _last_in_maps = in_maps
    from concourse import bass_utils
    res = bass_utils.run_bass_kernel_spmd(nc, in_maps,
                                          core_ids=list(range(NCORES)))
    out = np.empty((NT, D), np.float32)
    for c in range(NCORES):
        out[shard_rows[c]] = res.results[c]["out_shard"]
    return out.reshape(B, S, D)
